# revision 5
# baseline (speedup 1.0000x reference)
"""Trainium2 Bass kernel for nn_SampleRepresentativeCalculator.

Shards the Z (band) axis across 8 NeuronCores (28 bands per core).

Numerics (validated bit-level vs the jax reference on host, rel ~6e-4):
  host sends step8 = e4m3(2*m+1), with sentinel step=2^-6 where m==0
    (quantizing the residual on a 2^-6 grid == keeping it, so the
     lossless m==0 branch needs no predicated copy).
  c    = RECIP2(step8)  1/step via bitnot seed + 2 Newton       [vector]
  r    = o - p                                                  [gpsimd]
  w    = r * c                                                  [vector]
  q    = QF4(w, step8) = rint(w) * step                         [vector]
  rs   = 1/sqrt(|q + 1e-4|)     (Abs_reciprocal_sqrt LUT)       [scalar]
  rf2  = (rs * sqrt(psi))^2 = psi/|q|'  (Square LUT)            [scalar]
  adj  = ADJT(q, rf2): num' = clamp(q,-th,th) - q;
         adj = (-phi/(th+eps))*q*[num'==0] + num'*rf2           [vector]
  bcf  = p + q;  bc16 = f16(bcf)                        [gpsimd + scalar]
  reps16 = f16(bcf + adj)                                [vector/gpsimd]
"""
import numpy as np
import ml_dtypes

import concourse.bass as bass
import concourse.tile as tile
from concourse import bacc, mybir
from concourse.bass_utils import run_bass_kernel_spmd
from concourse.dve_ops import (
    DveOp, OPS, CUSTOM_DVE_SPECS, _SUB_OPCODE_FOR_NAME, _CUSTOM_DVE_ROW_BASE,
    has_src1,
)
from concourse.dve_spec import (
    Spec, Src0, Src1, C0, C1, C2, Zero, One, lower, maxx, minn, eq, Bin, AluOp,
)
from concourse.dve_uop import DveOpSpec

F32 = np.float32
F16 = np.float16
F8 = ml_dtypes.float8_e4m3

MAGIC = 12582912.0        # 1.5*2^23: rint(x) == (x+M)-M for |x| < 2^22
DELTA = float(2.0 ** -6)  # sentinel step for m==0
RC0, RC1 = -0.2355, 2.00175     # RECIP2 seed/newton consts (tuned on host)
EPS_RS = 1e-4

Z, Y, X = 224, 256, 512
N_CORES = 8
ZPC = Z // N_CORES          # 28 bands per core
FD = 2048                   # free dim per tile
ROWS = ZPC * Y * X // FD    # 1792 rows per core
N_TILES = ROWS // 128       # 14 tiles [128, FD]
BANDS_PER_TILE = 128 * FD // (Y * X)   # 2
PART_PER_BAND = 128 // BANDS_PER_TILE  # 64


def _register(name, spec, subdim=False):
    """Runtime-register a custom DVE op (mirrors DveOp.compile sha pinning)."""
    if name in _SUB_OPCODE_FOR_NAME:
        for op in OPS:
            if op.name == name:
                return op
        raise RuntimeError(name)
    opcode = _CUSTOM_DVE_ROW_BASE + len(OPS)
    assert opcode < 0x20, "custom DVE row overflow"
    shas = {}
    for ver in ("v3", "v4"):
        s = DveOpSpec(name=name, opcode=opcode, uops=lower(spec, ver=ver),
                      rd1_en=has_src1(spec))
        shas[ver] = s.sha(ver)
    op = DveOp(name, spec, subdim=subdim, uops_sha=shas)
    OPS.append(op)
    CUSTOM_DVE_SPECS[name] = spec
    _SUB_OPCODE_FOR_NAME[name] = opcode
    return op


def _bitnot_f32(x):
    x = np.ascontiguousarray(x, F32)
    return (~x.view(np.int32)).view(F32)


def _f32(x):
    return np.asarray(x, F32)


def _ref_recip2(in0, in1, c0, c1, c2):
    x = _f32(in0)
    n = _bitnot_f32(x)
    y0 = _f32(n * F32(c0))
    t0 = _f32(x * y0)
    u0 = _f32(F32(c1) - t0)
    y1 = _f32(y0 * u0)
    t1 = _f32(x * y1)
    u1 = _f32(F32(2.0) - t1)
    return _f32(y1 * u1)


def _ref_qf4(in0, in1, c0, c1, c2):
    w, s = _f32(in0), _f32(in1)
    a = _f32(w + F32(c0))
    k = _f32(a - F32(c0))
    return _f32(k * s)


def _ref_adjt(in0, in1, c0, c1, c2):
    q, rf2 = _f32(in0), _f32(in1)
    aa = np.maximum(q, F32(c1)).astype(F32)
    x2 = np.minimum(aa, F32(c2)).astype(F32)
    num = _f32(x2 - q)
    g = (num == 0).astype(F32)
    b1m = _f32(_f32(q * c0) * g)
    b2 = _f32(num * rf2)
    return _f32(b1m + b2)


# RECIP2: c = 2-Newton reciprocal of Src0 (seed = bitnot trick)
_n = Bin(AluOp.BITWISE_NOT, Src0, Src0)
_y0 = _n * C0
_y1 = _y0 * (C1 - Src0 * _y0)
_y2 = _y1 * ((One + One) - Src0 * _y1)
RECIP2_OP = _register("RECIP2_ANT", Spec(body=_y2, reference=_ref_recip2))

# QF4: q = rint(Src0) * Src1;  C0=magic, Src1 = step (f8 read-converted)
QF4_OP = _register(
    "QF4_ANT", Spec(body=((Src0 + C0) - C0) * Src1, reference=_ref_qf4))

# ADJT: adj from q (Src0) and rf2 = psi/den (Src1); C0=-phi', C1=-th, C2=th
_x2 = minn(maxx(Src0, C1), C2)
_num = _x2 - Src0          # = -num_q: vanishes iff |q| <= th
_adj = (Src0 * C0) * eq(_num, Zero) + _num * Src1
ADJT_OP = _register("ADJT_ANT", Spec(body=_adj, reference=_ref_adjt))


def build_kernel(th, bufs=2, n_reps_g=3, dma_spread=True):
    """n_reps_g: how many of every 4 tiles run the reps16 add on gpsimd."""
    nc = bacc.Bacc(
        "TRN2",
        target_bir_lowering=False,
        debug=False,
        enable_asserts=False,
        num_devices=N_CORES,
    )
    f32 = mybir.dt.float32
    f16 = mybir.dt.float16
    f8 = mybir.dt.float8e4
    o_d = nc.dram_tensor("o", [ROWS, FD], f32, kind="ExternalInput")
    p_d = nc.dram_tensor("p", [ROWS, FD], f32, kind="ExternalInput")
    s_d = nc.dram_tensor("s8", [ROWS, FD], f8, kind="ExternalInput")
    ps_d = nc.dram_tensor("ps", [128, 2 * N_TILES], f32, kind="ExternalInput")
    bc_d = nc.dram_tensor("bc16", [ROWS, FD], f16, kind="ExternalOutput")
    reps_d = nc.dram_tensor("reps16", [ROWS, FD], f16, kind="ExternalOutput")

    o_t = o_d[:].rearrange("(t p) f -> t p f", p=128)
    p_t = p_d[:].rearrange("(t p) f -> t p f", p=128)
    s_t = s_d[:].rearrange("(t p) f -> t p f", p=128)
    bc_t = bc_d[:].rearrange("(t p) f -> t p f", p=128)
    reps_t = reps_d[:].rearrange("(t p) f -> t p f", p=128)

    act = mybir.ActivationFunctionType
    deng = nc.scalar if dma_spread else nc.sync

    with tile.TileContext(nc) as tc:
        with tc.tile_pool(name="consts", bufs=1) as cpool, \
             tc.tile_pool(name="io", bufs=bufs) as iop, \
             tc.tile_pool(name="tmp", bufs=bufs) as tp:
            ps = cpool.tile([128, 2 * N_TILES], f32)
            nc.sync.dma_start(ps[:], ps_d[:])
            epsb = cpool.tile([128, 1], f32)
            nc.gpsimd.memset(epsb[:], EPS_RS)

            for t in range(N_TILES):
                ot = iop.tile([128, FD], f32, tag="o")
                pt = iop.tile([128, FD], f32, tag="p")
                st = iop.tile([128, FD], f8, tag="s8")
                nc.sync.dma_start(ot[:], o_t[t])
                deng.dma_start(pt[:], p_t[t])
                nc.sync.dma_start(st[:], s_t[t])

                phi_ap = ps[:, t:t + 1]                      # -phi/(th+eps)
                sz_ap = ps[:, N_TILES + t:N_TILES + t + 1]   # sqrt(psi)

                c = tp.tile([128, FD], f32, tag="c")
                nc.vector._custom_dve(RECIP2_OP, out=c[:], in0=st[:],
                                      s0=RC0, s1=RC1)

                r = tp.tile([128, FD], f32, tag="r")
                nc.gpsimd.tensor_sub(r[:], ot[:], pt[:])

                w = tp.tile([128, FD], f32, tag="w")
                nc.vector.tensor_mul(w[:], r[:], c[:])

                q = tp.tile([128, FD], f32, tag="q")
                nc.vector._custom_dve(QF4_OP, out=q[:], in0=w[:], in1=st[:],
                                      s0=MAGIC)

                rs = tp.tile([128, FD], f32, tag="rs")
                nc.scalar.activation(rs[:], q[:], act.Abs_reciprocal_sqrt,
                                     bias=epsb[:])

                rf2 = tp.tile([128, FD], f32, tag="rf2")
                nc.scalar.activation(rf2[:], rs[:], act.Square, scale=sz_ap)

                adj = tp.tile([128, FD], f32, tag="adj")
                nc.vector._custom_dve(ADJT_OP, out=adj[:], in0=q[:],
                                      in1=rf2[:], s0=phi_ap, s1=-th, imm2=th)

                bcf = tp.tile([128, FD], f32, tag="bcf")
                nc.gpsimd.tensor_add(bcf[:], pt[:], q[:])

                bc16 = tp.tile([128, FD], f16, tag="bc16")
                nc.scalar.activation(bc16[:], bcf[:], act.Copy)
                deng.dma_start(bc_t[t], bc16[:])

                rep16 = tp.tile([128, FD], f16, tag="rep16")
                if (t % 4) < n_reps_g:
                    nc.gpsimd.tensor_add(rep16[:], bcf[:], adj[:])
                else:
                    nc.vector.tensor_add(rep16[:], bcf[:], adj[:])
                deng.dma_start(reps_t[t], rep16[:])
    nc.compile()
    return nc


_NC_CACHE = {}
_BUILD_KW = {}


def _get_nc(th):
    key = (float(th), tuple(sorted(_BUILD_KW.items())))
    if key not in _NC_CACHE:
        _NC_CACHE[key] = build_kernel(th, **_BUILD_KW)
    return _NC_CACHE[key]


def kernel(original_samples, predicted_samples, max_errors, phi, psi, theta,
           _run_kwargs=None, _return_raw=False):
    o = np.ascontiguousarray(original_samples, F32)
    p = np.ascontiguousarray(predicted_samples, F32)
    mi = np.ascontiguousarray(max_errors, np.int32)
    phi = np.asarray(phi, F32)
    psi = np.asarray(psi, F32)
    th = float(np.asarray(theta, F32).reshape(-1)[0])

    # host-side lossless re-encode: step in {2m+1}, sentinel 2^-6 for m==0
    step = np.where(mi == 0, F32(DELTA), (2 * mi + 1).astype(F32)).astype(F8)

    the = (F32(th) + F32(1e-8)).astype(F32)
    phi_c = (-(phi / the)).astype(F32)
    psi_s = np.sqrt(psi).astype(F32)
    in_maps = []
    for i in range(N_CORES):
        z0 = i * ZPC
        ps = np.empty((128, 2 * N_TILES), F32)
        ps[:, :N_TILES] = np.repeat(
            phi_c[z0:z0 + ZPC].reshape(N_TILES, BANDS_PER_TILE).T,
            PART_PER_BAND, axis=0)
        ps[:, N_TILES:] = np.repeat(
            psi_s[z0:z0 + ZPC].reshape(N_TILES, BANDS_PER_TILE).T,
            PART_PER_BAND, axis=0)
        in_maps.append(dict(
            o=o[z0:z0 + ZPC].reshape(ROWS, FD),
            p=p[z0:z0 + ZPC].reshape(ROWS, FD),
            s8=step[z0:z0 + ZPC].reshape(ROWS, FD),
            ps=ps,
        ))

    nc = _get_nc(th)
    res = run_bass_kernel_spmd(nc, in_maps, list(range(N_CORES)),
                               **(_run_kwargs or {}))

    reps = np.empty((Z, Y, X), F32)
    bc = np.empty((Z, Y, X), F32)
    for i in range(N_CORES):
        z0 = i * ZPC
        reps[z0:z0 + ZPC] = res.results[i]["reps16"].astype(F32).reshape(ZPC, Y, X)
        bc[z0:z0 + ZPC] = res.results[i]["bc16"].astype(F32).reshape(ZPC, Y, X)
    if _return_raw:
        return (reps, bc), res
    return reps, bc
